# revision 1
# baseline (speedup 1.0000x reference)
"""Trainium2 Bass kernel for a pre-LN transformer block (MHA + MLP).

Strategy:
  - Data-parallel over batch: 32 batches -> 4 per core x 8 cores.
  - Everything on-device runs in "transposed" layout [C, T] so that all
    matmul contractions have their contraction dim on partitions. The host
    transposes x -> x^T before upload and the output back after download.
  - LayerNorm stats are computed with a ones-matrix matmul (M=128) so the
    per-token sums arrive in PSUM already replicated across partitions;
    no cross-partition broadcast is ever needed.
  - Softmax runs on S^T tiles [s, t]: exp on ScalarE (fused 1/sqrt(hs)
    scale), causal masking via a precomputed triangular mask on the
    diagonal blocks only, and the denominators come for free from 64
    ones-columns appended to V in the P^T @ [V|1] matmul.
  - Precision split: residual stream / LN stats / q,k and the scores
    matmul stay float32r (full fp32 bits in SBUF, FP22 in the PE, which
    keeps softmax logits accurate); the bulky matmuls (P@V, Wo, MLP) run
    in bf16, which streams 2x faster through the PE and enables fast
    weight loads.
  - PSUM: all accumulators are [128, 768] (2 banks); two tags x bufs=2
    fill the 8 banks and every phase needs at most 4 such tiles.
"""

import numpy as np
import ml_dtypes

import concourse.bass as bass
import concourse.mybir as mybir
import concourse.tile as tile
from concourse.bass_utils import run_bass_kernel_spmd

# ---- problem constants (hardcoded per harness contract) ----
B = 32
T = 768
C = 256
H = 4
HS = 64  # head size
F = 4 * C  # 1024
N_CORES = 8
B_PER_CORE = B // N_CORES  # 4
LN_EPS = 1e-5
F32 = mybir.dt.float32
F32R = mybir.dt.float32r
BF16 = mybir.dt.bfloat16

AF = mybir.ActivationFunctionType
ALU = mybir.AluOpType


def chunks512(lo, hi):
    """Chunk [lo, hi) into pieces of at most 512 (fp32 moving-operand cap),
    keeping every piece >= 256 wide when possible."""
    out = []
    while hi - lo > 512:
        ln = 512 if hi - lo >= 768 else hi - lo - 256
        out.append((lo, ln))
        lo += ln
    out.append((lo, hi - lo))
    return out


# This walrus build rejects >1 sem wait per instruction (setupSyncWait
# "Too many sync wait commands"). Post-pass: move excess waits onto
# freshly inserted same-engine NoOps immediately before the offender.
_MAX_WAITS = 1


def _split_waits(nc):
    n_new = 0
    for bass_bb in nc.bb_map.values():
        bb = bass_bb.bb
        insts = list(bb.instructions)
        out = []
        changed = False
        for inst in insts:
            si = getattr(inst, "sync_info", None)
            waits = list(si.on_wait) if si and si.on_wait else []
            if len(waits) > _MAX_WAITS:
                changed = True
                excess, keep = waits[:-_MAX_WAITS], waits[-_MAX_WAITS:]
                for j in range(0, len(excess), _MAX_WAITS):
                    nop = mybir.InstNoOp(name=f"waitnop-{n_new}", ins=[], outs=[])
                    n_new += 1
                    nop.engine = inst.engine
                    nop.sync_info = mybir.SyncInfo(
                        on_wait=excess[j:j + _MAX_WAITS], on_update=[])
                    out.append(nop)
                inst.sync_info = mybir.SyncInfo(
                    on_wait=keep, on_update=list(si.on_update))
            out.append(inst)
        if changed:
            bb.instructions = out
    return n_new


def _build_nc():
    nc = bass.Bass("TRN2", target_bir_lowering=False, debug=False,
                   num_devices=N_CORES)

    # ---- DRAM parameters ----
    P = nc.declare_dram_parameter
    xt_d = P("xt", [B_PER_CORE, C, T], F32R, isOutput=False)
    wq_d = P("wq", [2, 128, C], F32R, isOutput=False)
    wk_d = P("wk", [2, 128, C], F32R, isOutput=False)
    wv_d = P("wv", [2, 128, C], F32R, isOutput=False)
    wo_d = P("wo", [2, 128, C], BF16, isOutput=False)
    w1_d = P("w1", [2, 128, F], BF16, isOutput=False)
    w2_d = P("w2", [8, 128, C], BF16, isOutput=False)
    bq_d = P("bq", [128, 2], F32, isOutput=False)
    bk_d = P("bk", [128, 2], F32, isOutput=False)
    bv_d = P("bv", [128, C], F32, isOutput=False)
    bo_d = P("bo", [1, C], BF16, isOutput=False)
    b1_d = P("b1", [128, 8], F32, isOutput=False)
    b2_d = P("b2", [1, C], BF16, isOutput=False)
    mask_d = P("mask", [128, 128], F32, isOutput=False)
    onc_d = P("ones_c", [128, 128], F32R, isOutput=False)
    ont_d = P("ones_t", [1, T], BF16, isOutput=False)
    onv_d = P("ones_va", [128, C], F32R, isOutput=False)
    yt_d = P("yt", [B_PER_CORE, C, T], F32, isOutput=True)
    x1_d = P("x1dbg", [B_PER_CORE, C, T], F32, isOutput=True)
    pt_d = P("ptdbg", [B_PER_CORE, H, 128, T], F32, isOutput=True)
    ot_d = P("otdbg", [B_PER_CORE, C, T], F32, isOutput=True)
    rb_d = P("rbdbg", [B_PER_CORE, H, 64, T], F32, isOutput=True)

    with tile.TileContext(nc) as tc:
        with (
            tc.tile_pool(name="consts", bufs=1) as consts,
            tc.tile_pool(name="work", bufs=2) as work,
            tc.tile_pool(name="psum", bufs=2, space="PSUM") as psum,
        ):
            _kernel_body(nc, consts, work, psum, xt_d, wq_d, wk_d, wv_d,
                         wo_d, w1_d, w2_d, bq_d, bk_d, bv_d, bo_d, b1_d,
                         b2_d, mask_d, onc_d, ont_d, onv_d, yt_d, x1_d, pt_d, ot_d, rb_d)
    _split_waits(nc)
    return nc


def _kernel_body(nc, consts, work, psum, xt_d, wq_d, wk_d, wv_d, wo_d, w1_d,
                 w2_d, bq_d, bk_d, bv_d, bo_d, b1_d, b2_d, mask_d, onc_d,
                 ont_d, onv_d, yt_d, x1_d, pt_d, ot_d, rb_d):
    # ---- load constants ----
    wq_sb = [consts.tile([128, C], F32R, tag=f"wq{i}", name=f"wq{i}") for i in range(2)]
    wk_sb = [consts.tile([128, C], F32R, tag=f"wk{i}", name=f"wk{i}") for i in range(2)]
    wv_sb = [consts.tile([128, C], F32R, tag=f"wv{i}", name=f"wv{i}") for i in range(2)]
    wo_sb = [consts.tile([128, C], BF16, tag=f"wo{i}", name=f"wo{i}") for i in range(2)]
    w1_sb = [consts.tile([128, F], BF16, tag=f"w1{i}", name=f"w1{i}") for i in range(2)]
    w2_sb = [consts.tile([128, C], BF16, tag=f"w2{i}", name=f"w2{i}") for i in range(8)]
    for kt in range(2):
        nc.sync.dma_start(out=wq_sb[kt], in_=wq_d[kt])
        nc.sync.dma_start(out=wk_sb[kt], in_=wk_d[kt])
        nc.sync.dma_start(out=wv_sb[kt], in_=wv_d[kt])
        nc.sync.dma_start(out=wo_sb[kt], in_=wo_d[kt])
        nc.sync.dma_start(out=w1_sb[kt], in_=w1_d[kt])
    for kt in range(8):
        nc.sync.dma_start(out=w2_sb[kt], in_=w2_d[kt])
    bq_sb = consts.tile([128, 2], F32, tag="bq")
    bk_sb = consts.tile([128, 2], F32, tag="bk")
    bv_sb = consts.tile([128, C], F32, tag="bv")
    bo_sb = consts.tile([1, C], BF16, tag="bo")
    b1_sb = consts.tile([128, 8], F32, tag="b1")
    b2_sb = consts.tile([1, C], BF16, tag="b2")
    mask_sb = consts.tile([128, 128], F32, tag="mask")
    nc.sync.dma_start(out=bq_sb, in_=bq_d[:, :])
    nc.sync.dma_start(out=bk_sb, in_=bk_d[:, :])
    nc.sync.dma_start(out=bv_sb, in_=bv_d[:, :])
    nc.sync.dma_start(out=bo_sb, in_=bo_d[:, :])
    nc.sync.dma_start(out=b1_sb, in_=b1_d[:, :])
    nc.sync.dma_start(out=b2_sb, in_=b2_d[:, :])
    nc.sync.dma_start(out=mask_sb, in_=mask_d[:, :])

    # f32r memset fails the walrus ISA check, so ones come from DRAM
    ones_stat = consts.tile([128, 128], F32R, tag="ones_stat")
    nc.sync.dma_start(out=ones_stat, in_=onc_d[:, :])
    ones_row = consts.tile([1, T], BF16, tag="ones_row")
    nc.sync.dma_start(out=ones_row, in_=ont_d[:, :])
    ones_va = consts.tile([128, C], F32R, tag="ones_va")
    nc.sync.dma_start(out=ones_va, in_=onv_d[:, :])
    eps_sb = consts.tile([128, 1], F32, tag="eps")
    nc.vector.memset(eps_sb, LN_EPS)

    def layer_norm(src_sb, tag, out_dt):
        """src_sb: 2x [128, T] f32r tiles (c on partitions). Returns 2x
        [128, T] out_dt tiles, zero mean/unit var per t-column."""
        sq = [work.tile([128, T], F32R, tag=f"ln_sq{ct}", bufs=1,
                        name=f"{tag}_sq{ct}") for ct in range(2)]
        for ct in range(2):
            nc.gpsimd.tensor_tensor(out=sq[ct], in0=src_sb[ct],
                                    in1=src_sb[ct], op=ALU.mult)
        ps_mu = psum.tile([128, T], F32, tag="pa", name=f"{tag}_mu")
        ps_ex2 = psum.tile([128, T], F32, tag="pb", name=f"{tag}_ex2")
        for ps, rhs in ((ps_mu, src_sb), (ps_ex2, sq)):
            for st, ln in chunks512(0, T):
                for kt in range(2):
                    nc.tensor.matmul(
                        ps[:, st:st + ln], ones_stat,
                        rhs[kt][:, st:st + ln],
                        start=(kt == 0), stop=(kt == 1),
                    )
        t2 = work.tile([128, T], F32, tag="ln_t2", bufs=1)
        alpha = work.tile([128, T], F32, tag="ln_al", bufs=1)
        nc.scalar.activation(out=t2, in_=ps_mu, func=AF.Square)
        nc.vector.tensor_tensor(out=t2, in0=ps_ex2, in1=t2, op=ALU.subtract)
        nc.scalar.activation(out=t2, in_=t2, func=AF.Sqrt, bias=eps_sb,
                             scale=1.0)
        nc.vector.reciprocal(out=alpha, in_=t2)
        out_sb = []
        for ct in range(2):
            h_sb = work.tile([128, T], out_dt, tag=f"ln_h{ct}", bufs=2,
                             name=f"{tag}_h{ct}")
            hf = work.tile([128, T], F32, tag=f"ln_hf{ct}", bufs=1,
                           name=f"{tag}_hf{ct}")
            nc.vector.tensor_tensor(out=hf, in0=src_sb[ct], in1=ps_mu,
                                    op=ALU.subtract)
            nc.vector.tensor_tensor(out=h_sb, in0=hf, in1=alpha,
                                    op=ALU.mult)
            out_sb.append(h_sb)
        return out_sb

    for b in range(B_PER_CORE):
        # ---- load x^T ----
        xt = [work.tile([128, T], F32R, tag=f"xt{ct}", bufs=2,
                        name=f"xt{ct}") for ct in range(2)]
        for ct in range(2):
            nc.sync.dma_start(out=xt[ct], in_=xt_d[b, ct * 128:(ct + 1) * 128, :])

        # ---- LN1 (keep f32r for accurate q/k logits) ----
        ht = layer_norm(xt, f"ln1_{b}", F32R)

        # ---- QKV projections ----
        qt, kt_s = [], []
        for name, w_sb, b_col, dst, ptag in (("q", wq_sb, bq_sb, qt, "pa"),
                                             ("k", wk_sb, bk_sb, kt_s, "pb")):
            for mt in range(2):
                sb = work.tile([128, T], F32R, tag=f"{name}t{mt}", bufs=2,
                               name=f"{name}t{mt}")
                ps = psum.tile([128, T], F32, tag=ptag, name=f"ps_{name}{mt}")
                for st, ln in chunks512(0, T):
                    for kt in range(2):
                        nc.tensor.matmul(
                            ps[:, st:st + ln],
                            w_sb[kt][:, mt * 128:(mt + 1) * 128],
                            ht[kt][:, st:st + ln],
                            start=(kt == 0), stop=(kt == 1),
                        )
                nc.scalar.activation(out=sb, in_=ps, func=AF.Identity,
                                     bias=b_col[:, mt:mt + 1], scale=1.0)
                dst.append(sb)

        # v, packed as [v_h | ones*64] per head: the 64 ones-columns make
        # the P^T matmul emit softmax denominators in rows 64:128.
        vaug = []
        for tt in range(6):
            ps = psum.tile([128, C], F32, tag="pa", name=f"ps_v{tt}")
            for kt in range(2):
                nc.tensor.matmul(
                    ps,
                    ht[kt][:, tt * 128:(tt + 1) * 128],
                    wv_sb[kt],
                    start=(kt == 0), stop=(kt == 1),
                )
            va = work.tile([128, H, 128], F32R, tag=f"vaug{tt}", bufs=1,
                           name=f"vaug{tt}")
            nc.vector.tensor_copy(
                out=va[:, :, 64:128],
                in_=ones_va.rearrange("p (h d) -> p h d", h=H))
            nc.vector.tensor_tensor(
                out=va[:, :, 0:64],
                in0=ps.rearrange("p (h d) -> p h d", h=H),
                in1=bv_sb.rearrange("p (h d) -> p h d", h=H),
                op=ALU.add,
            )
            vaug.append(va)

        # ---- attention (per head) ----
        ot = [work.tile([128, T], BF16, tag=f"ot{mt}", bufs=2,
                        name=f"ot{mt}") for mt in range(2)]
        for h in range(H):
            mt, off = h // 2, (h % 2) * 64
            q_ap = qt[mt][off:off + 64, :]
            k_ap = kt_s[mt][off:off + 64, :]
            po = psum.tile([128, T], F32, tag="pa", name=f"ps_po{h}")
            pt = work.tile([128, T], F32R, tag="pt", bufs=3)
            for si in range(6):
                lo = si * 128
                ps_s = psum.tile([128, T - lo], F32, tag="pb",
                                 name=f"ps_s{h}_{si}")
                for st, ln in chunks512(lo, T):
                    nc.tensor.matmul(
                        ps_s[:, st - lo:st - lo + ln],
                        k_ap[:, lo:lo + 128],
                        q_ap[:, st:st + ln],
                        start=True, stop=True,
                    )
                nc.scalar.activation(out=pt[:, lo:T], in_=ps_s,
                                     func=AF.Exp, scale=HS ** -0.5)
                # causal mask on the diagonal block
                nc.vector.tensor_tensor(out=pt[:, lo:lo + 128],
                                        in0=pt[:, lo:lo + 128],
                                        in1=mask_sb, op=ALU.mult)
                for st, ln in chunks512(lo, T):
                    nc.tensor.matmul(
                        po[:, st:st + ln],
                        vaug[si][:, h, :],
                        pt[:, st:st + ln],
                        start=(si == 0), stop=(si == 5),
                    )
            nc.sync.dma_start(out=pt_d[b, h], in_=pt.bitcast(F32))
            # normalize: o / l  (rows 64:128 of po are l replicated)
            rb = work.tile([64, T], F32, tag="rb", bufs=2)
            nc.vector.reciprocal(out=rb, in_=po[64:128, :])
            nc.vector.tensor_tensor(out=ot[mt][off:off + 64, :],
                                    in0=po[0:64, :], in1=rb, op=ALU.mult)
            nc.sync.dma_start(out=rb_d[b, h], in_=rb)
            nc.gpsimd.dma_start(out=ot_d[b, (h % 2) * 64 + mt * 128:(h % 2) * 64 + mt * 128 + 64, :],
                                in_=ot[mt][off:off + 64, :])

        # ---- output projection + residual (+bo as rank-1 outer product) ----
        x1 = [work.tile([128, T], F32R, tag=f"x1_{ct}", bufs=2,
                        name=f"x1_{ct}") for ct in range(2)]
        for mt in range(2):
            ps = psum.tile([128, T], F32, tag="pb", name=f"ps_r{mt}")
            for st, ln in chunks512(0, T):
                for kt in range(2):
                    nc.tensor.matmul(
                        ps[:, st:st + ln],
                        wo_sb[kt][:, mt * 128:(mt + 1) * 128],
                        ot[kt][:, st:st + ln],
                        start=(kt == 0), stop=False,
                    )
                nc.tensor.matmul(
                    ps[:, st:st + ln],
                    bo_sb[0:1, mt * 128:(mt + 1) * 128],
                    ones_row[:, st:st + ln],
                    start=False, stop=True,
                )
            nc.vector.tensor_tensor(out=x1[mt], in0=ps, in1=xt[mt],
                                    op=ALU.add)
            nc.sync.dma_start(out=x1_d[b, mt * 128:(mt + 1) * 128, :],
                              in_=x1[mt].bitcast(F32))

        # ---- LN2 (bf16 is fine for the MLP) ----
        h2 = layer_norm(x1, f"ln2_{b}", BF16)

        # ---- MLP ----
        ps_y = [psum.tile([128, T], F32, tag="pa", name=f"ps_y{mt}")
                for mt in range(2)]
        for f in range(8):
            ut = work.tile([128, T], BF16, tag="ut", bufs=3)
            ps_u = psum.tile([128, T], F32, tag="pb", name=f"ps_u{f}")
            for st, ln in chunks512(0, T):
                for kt in range(2):
                    nc.tensor.matmul(
                        ps_u[:, st:st + ln],
                        w1_sb[kt][:, f * 128:(f + 1) * 128],
                        h2[kt][:, st:st + ln],
                        start=(kt == 0), stop=(kt == 1),
                    )
            nc.scalar.activation(out=ut, in_=ps_u, func=AF.Relu,
                                 bias=b1_sb[:, f:f + 1], scale=1.0)
            for mt in range(2):
                for st, ln in chunks512(0, T):
                    nc.tensor.matmul(
                        ps_y[mt][:, st:st + ln],
                        w2_sb[f][:, mt * 128:(mt + 1) * 128],
                        ut[:, st:st + ln],
                        start=(f == 0), stop=False,
                    )
        yt = [work.tile([128, T], F32, tag=f"yt{ct}", bufs=2,
                        name=f"yt{ct}") for ct in range(2)]
        for mt in range(2):
            for st, ln in chunks512(0, T):
                nc.tensor.matmul(
                    ps_y[mt][:, st:st + ln],
                    b2_sb[0:1, mt * 128:(mt + 1) * 128],
                    ones_row[:, st:st + ln],
                    start=False, stop=True,
                )
            nc.vector.tensor_tensor(out=yt[mt], in0=ps_y[mt], in1=x1[mt],
                                    op=ALU.add)
            nc.sync.dma_start(out=yt_d[b, mt * 128:(mt + 1) * 128, :],
                              in_=yt[mt])


_NC_CACHE = None


def _prep_weights(Wq, Wk, Wv, Wo, bo, W1, b1, W2, b2, g1, be1, g2, be2):
    f64 = np.float64
    g1, be1 = g1.astype(f64), be1.astype(f64)
    g2, be2 = g2.astype(f64), be2.astype(f64)

    def fold_qkv(W):  # W: [H, C, HS] -> folded [C, H*HS], bias [H*HS]
        Wf = W.astype(f64) * g1[None, :, None]
        Wcat = np.concatenate([Wf[h] for h in range(H)], axis=1)  # [C, 256]
        bias = np.concatenate([be1 @ Wf[h] for h in range(H)])  # [256]
        return Wcat, bias

    WqF, bq = fold_qkv(Wq)
    WkF, bk = fold_qkv(Wk)
    WvF, bv = fold_qkv(Wv)
    # h2 = z*g2 + be2 ; relu(h2@W1 + b1) = relu(z @ (g2*W1) + (be2@W1 + b1))
    W1F = W1.astype(f64) * g2[:, None]
    b1F = b1.astype(f64) + be2 @ W1.astype(f64)

    def f32(a):
        return np.ascontiguousarray(a, dtype=np.float32)

    def bf16(a):
        return np.ascontiguousarray(np.asarray(a, f64).astype(ml_dtypes.bfloat16))

    return {
        "wq": f32(WqF.reshape(2, 128, C)),
        "wk": f32(WkF.reshape(2, 128, C)),
        "wv": f32(WvF.reshape(2, 128, C)),
        "wo": bf16(np.asarray(Wo, f64).reshape(2, 128, C)),
        "w1": bf16(W1F.reshape(2, 128, F)),
        "w2": bf16(np.asarray(W2, f64).reshape(8, 128, C)),
        "bq": f32(bq.reshape(2, 128).T),
        "bk": f32(bk.reshape(2, 128).T),
        "bv": f32(np.broadcast_to(bv, (128, C))),
        "bo": bf16(np.asarray(bo, f64).reshape(1, C)),
        "b1": f32(b1F.reshape(8, 128).T),
        "b2": bf16(np.asarray(b2, f64).reshape(1, C)),
        "mask": f32(np.triu(np.ones((128, 128)))),
        "ones_c": f32(np.full((128, 128), 1.0 / C)),
        "ones_t": bf16(np.ones((1, T))),
        "ones_va": f32(np.ones((128, C))),
    }


def kernel(x, Wq, Wk, Wv, Wo, bo, W1, b1, W2, b2, g1, be1, g2, be2,
           _trace=False):
    global _NC_CACHE
    if _NC_CACHE is None:
        _NC_CACHE = _build_nc()
    nc = _NC_CACHE

    x = np.asarray(x, dtype=np.float32)
    weights = _prep_weights(
        np.asarray(Wq), np.asarray(Wk), np.asarray(Wv), np.asarray(Wo),
        np.asarray(bo), np.asarray(W1), np.asarray(b1), np.asarray(W2),
        np.asarray(b2), np.asarray(g1), np.asarray(be1), np.asarray(g2),
        np.asarray(be2))
    xt = np.ascontiguousarray(x.transpose(0, 2, 1))  # [B, C, T]

    in_maps = []
    for core in range(N_CORES):
        m = dict(weights)
        m["xt"] = np.ascontiguousarray(
            xt[core * B_PER_CORE:(core + 1) * B_PER_CORE])
        in_maps.append(m)

    res = run_bass_kernel_spmd(nc, in_maps, list(range(N_CORES)),
                               trace=_trace)
    outs = [res.results[i]["yt"] for i in range(N_CORES)]  # [4, C, T] each
    y = np.concatenate(outs, axis=0).transpose(0, 2, 1)  # [B, T, C]
    if _trace:
        kernel.last_exec_time_ns = res.exec_time_ns
        kernel.last_results = res
    return np.ascontiguousarray(y)



# revision 5
# speedup vs baseline: 1.6027x; 1.6027x over previous
"""Trainium2 Bass kernel for a pre-LN transformer block (MHA + MLP).

Strategy (v2):
  - Data-parallel over batch: 32 batches -> 4 per core x 8 cores.
  - Transposed layout [C, T] on device; host transposes in/out.
  - Phase-major schedule: each phase (LN1/QKV/ATTN/WO/LN2/MLP) runs for
    all 4 batch items before the next, so the scalar engine's activation
    table loads happen 3x per run instead of 2x per batch, and the PE
    always has cross-batch matmul work (stays at the warm 2.4 GHz clock).
  - LayerNorm: stats via ones-matmul (PSUM-replicated), var -> sqrt on
    ScalarE -> reciprocal_approx_fast on DVE (5x faster than the exact
    iterative reciprocal). Apply as h = x*alpha - beta (beta = mu*alpha)
    so the two big elementwise ops can run on GpSimd (no PSUM operand).
  - Attention: head pairs (partitions 0:64 / 64:128) issue score matmuls
    to disjoint PE row groups -> they run concurrently in the array.
    Scores for both heads land in one [128, 2, 384] PSUM tile; a single
    Exp call covers the pair. Causal masking is an extra matmul that
    accumulates -1e30 into the diagonal block (maskT x identity), so
    exp() underflows to exact zero - no elementwise mask op at all.
    Softmax denominators ride along as 64 ones-columns in the PV
    stationary; normalize via reciprocal_approx_fast + one multiply.
  - Residuals updated in-place in the xt tiles (f32 accuracy kept).
  - Dtypes: residual/LN stats f32r; q/k/ht/weights bf16 (fast weight
    load); exp'd scores + V in f32r (full softmax accuracy).
"""

import numpy as np
import ml_dtypes

import concourse.bass as bass
import concourse.mybir as mybir
import concourse.tile as tile
from concourse.bass_utils import run_bass_kernel_spmd

# ---- problem constants (hardcoded per harness contract) ----
B = 32
T = 768
C = 256
H = 4
HS = 64  # head size
F = 4 * C  # 1024
N_CORES = 8
B_PER_CORE = B // N_CORES  # 4
LN_EPS = 1e-5
F32 = mybir.dt.float32
F32R = mybir.dt.float32r
BF16 = mybir.dt.bfloat16

AF = mybir.ActivationFunctionType
ALU = mybir.AluOpType

NEG_BIG = -1e30


def chunks(lo, hi, cap):
    """Greedy split of [lo, hi) into pieces of at most cap."""
    out = []
    while lo < hi:
        ln = min(cap, hi - lo)
        out.append((lo, ln))
        lo += ln
    return out


# Attention query-chunk units per key-block si: (si, qlo, width).
# Width cap 512: the [128, 2, 512] pair score tile is exactly 2 PSUM
# banks, one bank per head, so each matmul output stays within a bank.
ATTN_UNITS = []
for _si in range(6):
    for _qlo, _w in chunks(_si * 128, T, 512):
        ATTN_UNITS.append((_si, _qlo, _w))


# This walrus build rejects >1 sem wait per instruction (setupSyncWait
# "Too many sync wait commands"). Post-pass: move excess waits onto
# freshly inserted same-engine NoOps immediately before the offender.
_MAX_WAITS = 1


def _split_waits(nc):
    n_new = 0
    for bass_bb in nc.bb_map.values():
        bb = bass_bb.bb
        insts = list(bb.instructions)
        out = []
        changed = False
        for inst in insts:
            si = getattr(inst, "sync_info", None)
            waits = list(si.on_wait) if si and si.on_wait else []
            if len(waits) > _MAX_WAITS:
                changed = True
                excess, keep = waits[:-_MAX_WAITS], waits[-_MAX_WAITS:]
                for j in range(0, len(excess), _MAX_WAITS):
                    nop = mybir.InstNoOp(name=f"waitnop-{n_new}", ins=[], outs=[])
                    n_new += 1
                    nop.engine = inst.engine
                    nop.sync_info = mybir.SyncInfo(
                        on_wait=excess[j:j + _MAX_WAITS], on_update=[])
                    out.append(nop)
                inst.sync_info = mybir.SyncInfo(
                    on_wait=keep, on_update=list(si.on_update))
            out.append(inst)
        if changed:
            bb.instructions = out
    return n_new


def _build_nc(has_qkb, has_vb, has_bo, has_b1, has_b2):
    nc = bass.Bass("TRN2", target_bir_lowering=False, debug=False,
                   num_devices=N_CORES)

    P = nc.declare_dram_parameter
    xt_d = P("xt", [B_PER_CORE, C, T], F32R, isOutput=False)
    wq_d = P("wq", [2, 128, C], BF16, isOutput=False)
    wk_d = P("wk", [2, 128, C], BF16, isOutput=False)
    wv_d = P("wv", [2, 128, C], BF16, isOutput=False)
    wo_d = P("wo", [2, 128, C], BF16, isOutput=False)
    w1_d = P("w1", [2, 128, F], BF16, isOutput=False)
    w2_d = P("w2", [8, 128, C], BF16, isOutput=False)
    bq_d = P("bq", [128, 2], F32, isOutput=False)
    bk_d = P("bk", [128, 2], F32, isOutput=False)
    bv_d = P("bv", [128, C], F32, isOutput=False)
    bo_d = P("bo", [1, C], BF16, isOutput=False)
    b1_d = P("b1", [128, 8], F32, isOutput=False)
    b2_d = P("b2", [1, C], BF16, isOutput=False)
    maskt_d = P("maskt", [128, 128], BF16, isOutput=False)
    ident_d = P("ident", [128, 128], BF16, isOutput=False)
    onc_d = P("ones_c", [128, 128], F32R, isOutput=False)
    onv_d = P("ones_va", [128, C], BF16, isOutput=False)
    ont_d = P("ones_t", [1, T], BF16, isOutput=False)
    yt_d = P("yt", [B_PER_CORE, C, T], F32, isOutput=True)

    with tile.TileContext(nc) as tc:
        with (
            tc.tile_pool(name="consts", bufs=1) as consts,
            tc.tile_pool(name="per_b", bufs=1) as per_b,
            tc.tile_pool(name="work", bufs=2) as work,
            tc.tile_pool(name="psum", bufs=2, space="PSUM") as psum,
        ):
            _kernel_body(
                nc, consts, per_b, work, psum,
                xt_d, wq_d, wk_d, wv_d, wo_d, w1_d, w2_d,
                bq_d, bk_d, bv_d, bo_d, b1_d, b2_d,
                maskt_d, ident_d, onc_d, onv_d, ont_d, yt_d,
                has_qkb, has_vb, has_bo, has_b1, has_b2,
            )
    _split_waits(nc)
    return nc


def _kernel_body(nc, consts, per_b, work, psum,
                 xt_d, wq_d, wk_d, wv_d, wo_d, w1_d, w2_d,
                 bq_d, bk_d, bv_d, bo_d, b1_d, b2_d,
                 maskt_d, ident_d, onc_d, onv_d, ont_d, yt_d,
                 has_qkb, has_vb, has_bo, has_b1, has_b2):
    NB = B_PER_CORE

    # ---- constants ----
    wq_sb = [consts.tile([128, C], BF16, tag=f"wq{i}", name=f"wq{i}") for i in range(2)]
    wk_sb = [consts.tile([128, C], BF16, tag=f"wk{i}", name=f"wk{i}") for i in range(2)]
    wv_sb = [consts.tile([128, C], BF16, tag=f"wv{i}", name=f"wv{i}") for i in range(2)]
    wo_sb = [consts.tile([128, C], BF16, tag=f"wo{i}", name=f"wo{i}") for i in range(2)]
    w1_sb = [consts.tile([128, F], BF16, tag=f"w1{i}", name=f"w1{i}") for i in range(2)]
    w2_sb = [consts.tile([128, C], BF16, tag=f"w2{i}", name=f"w2{i}") for i in range(8)]
    for kt in range(2):
        nc.sync.dma_start(out=wq_sb[kt], in_=wq_d[kt])
        nc.sync.dma_start(out=wk_sb[kt], in_=wk_d[kt])
        nc.sync.dma_start(out=wv_sb[kt], in_=wv_d[kt])
        nc.sync.dma_start(out=wo_sb[kt], in_=wo_d[kt])
        nc.sync.dma_start(out=w1_sb[kt], in_=w1_d[kt])
    for kt in range(8):
        nc.sync.dma_start(out=w2_sb[kt], in_=w2_d[kt])

    maskt_sb = consts.tile([128, 128], BF16, tag="maskt")
    ident_sb = consts.tile([128, 128], BF16, tag="ident")
    ones_stat = consts.tile([128, 128], F32R, tag="ones_stat")
    ones_va = consts.tile([128, C], BF16, tag="ones_va")
    nc.sync.dma_start(out=maskt_sb, in_=maskt_d[:, :])
    nc.sync.dma_start(out=ident_sb, in_=ident_d[:, :])
    nc.sync.dma_start(out=ones_stat, in_=onc_d[:, :])
    nc.sync.dma_start(out=ones_va, in_=onv_d[:, :])
    eps_sb = consts.tile([128, 1], F32, tag="eps")
    nc.vector.memset(eps_sb, LN_EPS)

    if has_qkb:
        bq_sb = consts.tile([128, 2], F32, tag="bq")
        bk_sb = consts.tile([128, 2], F32, tag="bk")
        nc.sync.dma_start(out=bq_sb, in_=bq_d[:, :])
        nc.sync.dma_start(out=bk_sb, in_=bk_d[:, :])
    if has_vb:
        bv_sb = consts.tile([128, C], F32, tag="bv")
        nc.sync.dma_start(out=bv_sb, in_=bv_d[:, :])
    if has_b1:
        b1_sb = consts.tile([128, 8], F32, tag="b1")
        nc.sync.dma_start(out=b1_sb, in_=b1_d[:, :])
    if has_bo or has_b2:
        ones_row = consts.tile([1, T], BF16, tag="ones_row")
        nc.sync.dma_start(out=ones_row, in_=ont_d[:, :])
        if has_bo:
            bo_sb = consts.tile([1, C], BF16, tag="bo")
            nc.sync.dma_start(out=bo_sb, in_=bo_d[:, :])
        if has_b2:
            b2_sb = consts.tile([1, C], BF16, tag="b2")
            nc.sync.dma_start(out=b2_sb, in_=b2_d[:, :])

    # ---- per-batch persistent tiles ----
    xt = [[per_b.tile([128, T], F32R, tag=f"xt{b}_{ct}", name=f"xt{b}_{ct}") for ct in range(2)]
          for b in range(NB)]
    ht = [[per_b.tile([128, T], BF16, tag=f"ht{b}_{ct}", name=f"ht{b}_{ct}") for ct in range(2)]
          for b in range(NB)]
    q_sb = [[per_b.tile([128, T], BF16, tag=f"q{b}_{mt}", name=f"q{b}_{mt}") for mt in range(2)]
            for b in range(NB)]
    k_sb = [[per_b.tile([128, T], BF16, tag=f"k{b}_{mt}", name=f"k{b}_{mt}") for mt in range(2)]
            for b in range(NB)]
    vaug = [[per_b.tile([128, H, 128], BF16, tag=f"va{b}_{tt}", name=f"va{b}_{tt}")
             for tt in range(6)] for b in range(NB)]
    ot = [[per_b.tile([128, T], BF16, tag=f"ot{b}_{mt}", name=f"ot{b}_{mt}") for mt in range(2)]
          for b in range(NB)]

    # ones halves of vaug: written once per run
    for b in range(NB):
        for tt in range(6):
            nc.vector.tensor_copy(
                out=vaug[b][tt][:, :, 64:128],
                in_=ones_va.rearrange("p (h d) -> p h d", h=H))

    # ---- load x^T ----
    for b in range(NB):
        for ct in range(2):
            nc.sync.dma_start(out=xt[b][ct],
                              in_=xt_d[b, ct * 128:(ct + 1) * 128, :])

    def layer_norm(b, src, out_tiles, tag):
        """src: 2x [128, T] f32r (c on partitions). Writes out_tiles (bf16)
        with zero-mean unit-var columns. h = x*alpha - beta."""
        sq = [work.tile([128, T], F32R, tag=f"ln_sq{ct}", bufs=2,
                        name=f"{tag}_sq{ct}") for ct in range(2)]
        for ct in range(2):
            nc.gpsimd.tensor_tensor(out=sq[ct], in0=src[ct], in1=src[ct],
                                    op=ALU.mult)
        ps_mu = psum.tile([128, T], F32, tag="pA", name=f"{tag}_mu")
        ps_ex2 = psum.tile([128, T], F32, tag="pB", name=f"{tag}_ex2")
        for ps, rhs in ((ps_mu, src), (ps_ex2, sq)):
            for kt in range(2):
                for st, ln in chunks(0, T, 512):
                    nc.tensor.matmul(
                        ps[:, st:st + ln], ones_stat, rhs[kt][:, st:st + ln],
                        start=(kt == 0), stop=(kt == 1))
        t2 = work.tile([128, T], F32, tag="ln_t2", bufs=2, name=f"{tag}_t2")
        alpha = work.tile([128, T], F32, tag="ln_al", bufs=2,
                          name=f"{tag}_al")
        beta = work.tile([128, T], F32, tag="ln_be", bufs=2,
                         name=f"{tag}_be")
        nc.scalar.activation(out=t2, in_=ps_mu, func=AF.Square)
        nc.vector.tensor_tensor(out=t2, in0=ps_ex2, in1=t2, op=ALU.subtract)
        # alpha = (var+eps)^-0.5 = exp(-0.5*ln(var+eps)); Ln/Exp share the
        # natural_log_exp table set with the attention exp (no table swap).
        nc.scalar.activation(out=t2, in_=t2, func=AF.Ln, bias=eps_sb,
                             scale=1.0)
        nc.scalar.activation(out=alpha, in_=t2, func=AF.Exp, scale=-0.5)
        nc.vector.tensor_tensor(out=beta, in0=ps_mu, in1=alpha, op=ALU.mult)
        for ct in range(2):
            g1 = work.tile([128, T], F32, tag=f"ln_g{ct}", bufs=2,
                           name=f"{tag}_g{ct}")
            nc.gpsimd.tensor_tensor(out=g1, in0=src[ct], in1=alpha,
                                    op=ALU.mult)
            nc.gpsimd.tensor_tensor(out=out_tiles[ct], in0=g1, in1=beta,
                                    op=ALU.subtract)

    # ================= LN1 =================
    for b in range(NB):
        layer_norm(b, xt[b], ht[b], f"ln1_{b}")

    # ================= QKV =================
    for b in range(NB):
        for name, w_sb, dst, b_sb in (("q", wq_sb, q_sb[b],
                                       bq_sb if has_qkb else None),
                                      ("k", wk_sb, k_sb[b],
                                       bk_sb if has_qkb else None)):
            for mt in range(2):
                ps = psum.tile([128, T], F32, tag="pA" if mt == 0 else "pB",
                               name=f"ps_{name}{b}_{mt}")
                for kt in range(2):
                    for st, ln in chunks(0, T, 512):
                        nc.tensor.matmul(
                            ps[:, st:st + ln],
                            w_sb[kt][:, mt * 128:(mt + 1) * 128],
                            ht[b][kt][:, st:st + ln],
                            start=(kt == 0), stop=(kt == 1))
                if has_qkb:
                    nc.scalar.activation(out=dst[mt], in_=ps,
                                         func=AF.Identity,
                                         bias=b_sb[:, mt:mt + 1], scale=1.0)
                else:
                    nc.vector.tensor_copy(out=dst[mt], in_=ps)
        for tt in range(6):
            ps = psum.tile([128, C], F32, tag="pA" if tt % 2 == 0 else "pB",
                           name=f"ps_v{b}_{tt}")
            for kt in range(2):
                nc.tensor.matmul(
                    ps, ht[b][kt][:, tt * 128:(tt + 1) * 128], wv_sb[kt],
                    start=(kt == 0), stop=(kt == 1))
            if has_vb:
                nc.vector.tensor_tensor(
                    out=vaug[b][tt][:, :, 0:64],
                    in0=ps.rearrange("p (h d) -> p h d", h=H),
                    in1=bv_sb.rearrange("p (h d) -> p h d", h=H),
                    op=ALU.add)
            else:
                nc.vector.tensor_copy(
                    out=vaug[b][tt][:, :, 0:64],
                    in_=ps.rearrange("p (h d) -> p h d", h=H))

    # ================= attention =================
    for b in range(NB):
        for mt in range(2):
            po = [psum.tile([128, T], F32, tag="pB", name=f"po{b}_{mt}_{hh}")
                  for hh in range(2)]
            pt = work.tile([128, 2, T], BF16, tag="ptp", bufs=2,
                           name=f"pt{b}_{mt}")
            for si, qlo, w in ATTN_UNITS:
                if qlo < si * 128:
                    continue
                diag = (qlo == si * 128)
                ps_s = psum.tile([128, 2, 512], F32, tag="pA",
                                 name=f"ps_s{b}_{mt}_{si}_{qlo}")
                for hh in range(2):
                    nc.tensor.matmul(
                        ps_s[:, hh, 0:w],
                        k_sb[b][mt][hh * 64:hh * 64 + 64,
                                    si * 128:si * 128 + 128],
                        q_sb[b][mt][hh * 64:hh * 64 + 64, qlo:qlo + w],
                        start=True, stop=not diag)
                if diag:
                    for hh in range(2):
                        nc.tensor.matmul(
                            ps_s[:, hh, 0:128], maskt_sb, ident_sb,
                            start=False, stop=True)
                nc.scalar.activation(out=pt[:, :, qlo:qlo + w],
                                     in_=ps_s[:, :, 0:w],
                                     func=AF.Exp, scale=HS ** -0.5)
                for hh in range(2):
                    nc.tensor.matmul(
                        po[hh][:, qlo:qlo + w],
                        vaug[b][si][:, 2 * mt + hh, :],
                        pt[:, hh, qlo:qlo + w],
                        start=(si == 0), stop=(si == 5))
            rb = work.tile([64, 2, T], F32, tag="rb", bufs=2,
                           name=f"rb{b}_{mt}")
            for hh in range(2):
                nc.scalar.activation(out=rb[:, hh, :], in_=po[hh][64:128, :],
                                     func=AF.Ln)
            nc.scalar.activation(out=rb, in_=rb, func=AF.Exp, scale=-1.0)
            for hh in range(2):
                nc.vector.tensor_tensor(
                    out=ot[b][mt][hh * 64:hh * 64 + 64, :],
                    in0=po[hh][0:64, :], in1=rb[:, hh, :], op=ALU.mult)

    # ================= Wo + residual (in-place into xt) =================
    for b in range(NB):
        for mt in range(2):
            ps = psum.tile([128, T], F32, tag="pA", name=f"ps_r{b}_{mt}")
            for kt in range(2):
                for st, ln in chunks(0, T, 512):
                    nc.tensor.matmul(
                        ps[:, st:st + ln],
                        wo_sb[kt][:, mt * 128:(mt + 1) * 128],
                        ot[b][kt][:, st:st + ln],
                        start=(kt == 0), stop=(kt == 1) and not has_bo)
            if has_bo:
                for st, ln in chunks(0, T, 512):
                    nc.tensor.matmul(
                        ps[:, st:st + ln],
                        bo_sb[0:1, mt * 128:(mt + 1) * 128],
                        ones_row[:, st:st + ln], start=False, stop=True)
            nc.vector.tensor_tensor(out=xt[b][mt], in0=ps, in1=xt[b][mt],
                                    op=ALU.add)

    # ================= LN2 =================
    h2 = ht  # reuse the ht tiles (dead after QKV)
    for b in range(NB):
        layer_norm(b, xt[b], h2[b], f"ln2_{b}")

    # ================= MLP =================
    for b in range(NB):
        ps_y = [psum.tile([128, T], F32, tag="pB", name=f"ps_y{b}_{mt}")
                for mt in range(2)]
        for f in range(8):
            ps_u = psum.tile([128, T], F32, tag="pA", name=f"ps_u{b}_{f}")
            for kt in range(2):
                for st, ln in chunks(0, T, 512):
                    nc.tensor.matmul(
                        ps_u[:, st:st + ln],
                        w1_sb[kt][:, f * 128:(f + 1) * 128],
                        h2[b][kt][:, st:st + ln],
                        start=(kt == 0), stop=(kt == 1))
            ut = work.tile([128, T], BF16, tag="ut", bufs=3,
                           name=f"ut{b}_{f}")
            if has_b1:
                nc.vector.tensor_scalar(out=ut, in0=ps_u,
                                        scalar1=b1_sb[:, f:f + 1],
                                        scalar2=0.0, op0=ALU.add,
                                        op1=ALU.max)
            else:
                nc.vector.tensor_scalar_max(out=ut, in0=ps_u, scalar1=0.0)
            for mt in range(2):
                for st, ln in chunks(0, T, 512):
                    nc.tensor.matmul(
                        ps_y[mt][:, st:st + ln],
                        w2_sb[f][:, mt * 128:(mt + 1) * 128],
                        ut[:, st:st + ln],
                        start=(f == 0), stop=(f == 7) and not has_b2)
        for mt in range(2):
            if has_b2:
                for st, ln in chunks(0, T, 512):
                    nc.tensor.matmul(
                        ps_y[mt][:, st:st + ln],
                        b2_sb[0:1, mt * 128:(mt + 1) * 128],
                        ones_row[:, st:st + ln], start=False, stop=True)
            nc.vector.tensor_tensor(out=xt[b][mt], in0=ps_y[mt],
                                    in1=xt[b][mt], op=ALU.add)
            nc.sync.dma_start(out=yt_d[b, mt * 128:(mt + 1) * 128, :],
                              in_=xt[b][mt].bitcast(F32))


_NC_CACHE = {}


def _prep_weights(Wq, Wk, Wv, Wo, bo, W1, b1, W2, b2, g1, be1, g2, be2):
    f64 = np.float64
    g1, be1 = g1.astype(f64), be1.astype(f64)
    g2, be2 = g2.astype(f64), be2.astype(f64)

    def fold_qkv(W):  # W: [H, C, HS] -> folded [C, H*HS], bias [H*HS]
        Wf = W.astype(f64) * g1[None, :, None]
        Wcat = np.concatenate([Wf[h] for h in range(H)], axis=1)  # [C, 256]
        bias = np.concatenate([be1 @ Wf[h] for h in range(H)])  # [256]
        return Wcat, bias

    WqF, bq = fold_qkv(Wq)
    WkF, bk = fold_qkv(Wk)
    WvF, bv = fold_qkv(Wv)
    W1F = W1.astype(f64) * g2[:, None]
    b1F = b1.astype(f64) + be2 @ W1.astype(f64)

    def f32(a):
        return np.ascontiguousarray(a, dtype=np.float32)

    def bf16(a):
        return np.ascontiguousarray(
            np.asarray(a, f64).astype(ml_dtypes.bfloat16))

    r = np.arange(128)
    maskt = np.where(r[None, :] <= r[:, None], 0.0, NEG_BIG)  # [k, m]

    return {
        "wq": bf16(WqF.reshape(2, 128, C)),
        "wk": bf16(WkF.reshape(2, 128, C)),
        "wv": bf16(WvF.reshape(2, 128, C)),
        "wo": bf16(np.asarray(Wo, f64).reshape(2, 128, C)),
        "w1": bf16(W1F.reshape(2, 128, F)),
        "w2": bf16(np.asarray(W2, f64).reshape(8, 128, C)),
        "bq": f32(bq.reshape(2, 128).T),
        "bk": f32(bk.reshape(2, 128).T),
        "bv": f32(np.broadcast_to(bv, (128, C))),
        "bo": bf16(np.asarray(bo, f64).reshape(1, C)),
        "b1": f32(b1F.reshape(8, 128).T),
        "b2": bf16(np.asarray(b2, f64).reshape(1, C)),
        "maskt": bf16(maskt),
        "ident": bf16(np.eye(128)),
        "ones_c": f32(np.full((128, 128), 1.0 / C)),
        "ones_va": bf16(np.ones((128, C))),
        "ones_t": bf16(np.ones((1, T))),
    }, (bq, bk, bv, b1F)


def kernel(x, Wq, Wk, Wv, Wo, bo, W1, b1, W2, b2, g1, be1, g2, be2,
           _trace=False):
    x = np.asarray(x, dtype=np.float32)
    weights, (bq, bk, bv, b1F) = _prep_weights(
        np.asarray(Wq), np.asarray(Wk), np.asarray(Wv), np.asarray(Wo),
        np.asarray(bo), np.asarray(W1), np.asarray(b1), np.asarray(W2),
        np.asarray(b2), np.asarray(g1), np.asarray(be1), np.asarray(g2),
        np.asarray(be2))

    flags = (bool(np.any(bq) or np.any(bk)), bool(np.any(bv)),
             bool(np.any(np.asarray(bo))), bool(np.any(b1F)),
             bool(np.any(np.asarray(b2))))
    if flags not in _NC_CACHE:
        _NC_CACHE[flags] = _build_nc(*flags)
    nc = _NC_CACHE[flags]

    xt = np.ascontiguousarray(x.transpose(0, 2, 1))  # [B, C, T]
    in_maps = []
    for core in range(N_CORES):
        m = dict(weights)
        m["xt"] = np.ascontiguousarray(
            xt[core * B_PER_CORE:(core + 1) * B_PER_CORE])
        in_maps.append(m)

    res = run_bass_kernel_spmd(nc, in_maps, list(range(N_CORES)),
                               trace=_trace)
    outs = [res.results[i]["yt"] for i in range(N_CORES)]  # [4, C, T] each
    y = np.concatenate(outs, axis=0).transpose(0, 2, 1)  # [B, T, C]
    if _trace:
        kernel.last_exec_time_ns = res.exec_time_ns
        kernel.last_results = res
    return np.ascontiguousarray(y)
